# revision 16
# baseline (speedup 1.0000x reference)
"""HGAT retrieval-kNN kernel for Trainium2, data-parallel over batch on 8 cores.

Reference semantics per batch element:
  pre = W @ x + b; pairwise = -||pre_v - pre_u||^2; idx = top_k(pairwise, 32)
  s[v,k] = q[(32v+k) % 1024] + r[idx[v,k]] with q = a1.pre, r = a2.pre
  H = softmax(s, axis=batch)

Device-side reduction (rank/softmax-equivalent):
  With M = W^T W, every bias term either is constant per top-k row or
  cancels:  z*[v,u] = x_v.(Mx)_u - 0.5 * x_u.(Mx)_u
  ranks identically to pairwise per row, and q' = (W^T a1).x,
  r' = (W^T a2).x differ from q, r by batch-independent constants that
  cancel in the softmax over the batch axis.  So the device needs only:
  Mx (K=64 matmul), xmx = x*Mx elementwise, one K=65 augmented matmul
  per 512-col half for z* (lhsT row 64 = ones, rhs row 64 =
  -0.5*colsum(xmx)), and the exact top-32 (tie-break identical to
  jax.lax.top_k): per round, DVE max8 + max_index; the removal step is
  offloaded from the DVE (the bottleneck engine) to ACT+GPSIMD as
  m = Sign(t - z), mb = Prelu(BIG*m - BIG, a=0.5) in {0,-BIG/2,-BIG},
  z += mb -- exactly 0 is added to survivors so their bits are
  untouched, and the >= t boundary handling matches match_replace.
  The chain runs on 512-col halves so it pipelines behind the next
  chunk's DVE work.
Host: gather r' by idx, add q', softmax over batch.
"""

import numpy as np

B, C_IN, V = 32, 64, 1024
C_REL, K = 128, 32
N_CORES = 8
BPC = B // N_CORES  # 4 batches per core

_cache = {}


def _build():
    import concourse.bacc as bacc
    import concourse.mybir as mybir
    import concourse.tile as tile

    dt = mybir.dt
    AF = mybir.ActivationFunctionType
    AO = mybir.AluOpType
    nc = bacc.Bacc(None, target_bir_lowering=False, debug=False)

    x_d = nc.dram_tensor("x", [BPC, C_IN, V], dt.float32, kind="ExternalInput")
    w_d = nc.dram_tensor("w", [C_REL, C_IN], dt.float32, kind="ExternalInput")
    a12_d = nc.dram_tensor("a12", [C_REL, 2], dt.float32, kind="ExternalInput")
    mi_d = nc.dram_tensor("mi", [BPC, 128, 256], dt.uint16, kind="ExternalOutput")
    qr_d = nc.dram_tensor("qr", [BPC, 2, V], dt.float32, kind="ExternalOutput")

    with tile.TileContext(nc) as tc:
        with tc.tile_pool(name="const", bufs=1) as cpool, \
             tc.tile_pool(name="perb", bufs=2) as bpool, \
             tc.tile_pool(name="zsb", bufs=8) as zpool, \
             tc.tile_pool(name="mvp", bufs=8) as mvpool, \
             tc.tile_pool(name="sgp", bufs=6) as sgpool, \
             tc.tile_pool(name="pszh", bufs=4, space="PSUM") as pszh, \
             tc.tile_pool(name="psm", bufs=2, space="PSUM") as psm, \
             tc.tile_pool(name="pss", bufs=2, space="PSUM") as pss:

            w_sb = cpool.tile([C_REL, C_IN], dt.float32)
            nc.sync.dma_start(w_sb[:], w_d[:])
            a12_sb = cpool.tile([C_REL, 2], dt.float32)
            nc.sync.dma_start(a12_sb[:], a12_d[:])
            mhalf = cpool.tile([C_IN, 1], dt.float32)
            nc.vector.memset(mhalf[:], -0.5)
            bigc = cpool.tile([128, 1], dt.float32)
            nc.vector.memset(bigc[:], 1.0e30)
            nbigc = cpool.tile([128, 1], dt.float32)
            nc.vector.memset(nbigc[:], -1.0e30)
            halfc = cpool.tile([128, 1], dt.float32)
            nc.vector.memset(halfc[:], 0.5)

            # M = W^T W [64,64]; wa = W^T [a1 a2] [64,2]  (one-time)
            m_sb = cpool.tile([C_IN, C_IN], dt.float32)
            wa_sb = cpool.tile([C_IN, 2], dt.float32)
            pm = psm.tile([65, 512], dt.float32, tag="pmx")
            nc.tensor.matmul(pm[0:C_IN, 0:C_IN], w_sb[:], w_sb[:, 0:C_IN],
                             start=True, stop=True)
            nc.scalar.copy(m_sb[:], pm[0:C_IN, 0:C_IN])
            pwa = psm.tile([65, 512], dt.float32, tag="pmx")
            nc.tensor.matmul(pwa[0:C_IN, 0:2], w_sb[:], a12_sb[:],
                             start=True, stop=True)
            nc.scalar.copy(wa_sb[:], pwa[0:C_IN, 0:2])

            for b in range(BPC):
                # xr: rows 0-63 = x, row 64 = -0.5*colsum(x*Mx)
                # mxl: rows 0-63 = Mx, row 64 = ones
                xr = bpool.tile([65, V], dt.float32, tag="xr")
                nc.sync.dma_start(xr[0:C_IN, 0:512], x_d[b][:, 0:512])
                nc.sync.dma_start(xr[0:C_IN, 512:1024], x_d[b][:, 512:1024])

                mxl = bpool.tile([65, V], dt.float32, tag="mxl")
                xmx = bpool.tile([C_IN, V], dt.float32, tag="xmx")
                qr_sb = bpool.tile([2, V], dt.float32, tag="qr")
                for h in range(2):
                    hs = slice(h * 512, (h + 1) * 512)
                    pmx = psm.tile([65, 512], dt.float32, tag="pmx")
                    nc.tensor.matmul(pmx[0:C_IN, :], m_sb[:], xr[0:C_IN, hs],
                                     start=True, stop=True)
                    nc.scalar.copy(mxl[0:C_IN, hs], pmx[0:C_IN, :])
                    # xmx = x * Mx (gpsimd, off the hot engines)
                    nc.gpsimd.tensor_tensor(out=xmx[:, hs], in0=xr[0:C_IN, hs],
                                            in1=mxl[0:C_IN, hs], op=AO.mult)
                    prow = pss.tile([2, 512], dt.float32, tag="pxs")
                    nc.tensor.matmul(prow[0:1, :], mhalf[:], xmx[:, hs],
                                     start=True, stop=True)
                    nc.scalar.copy(xr[64:65, hs], prow[0:1, :])
                    # q', r'
                    pqr = pss.tile([2, 512], dt.float32, tag="pxs")
                    nc.tensor.matmul(pqr[:], wa_sb[:], xr[0:C_IN, hs],
                                     start=True, stop=True)
                    nc.scalar.copy(qr_sb[:, hs], pqr[:])
                # mxl row 64 = 1.0 (Copy(in*0 + 1); input is arbitrary)
                nc.scalar.activation(mxl[64:65, :], xmx[0:1, :],
                                     AF.Copy, bias=1.0, scale=0.0)
                nc.sync.dma_start(qr_d[b], qr_sb[:])

                mi_sb = bpool.tile([128, 256], dt.uint16, tag="mi")
                for c in range(8):
                    # z*[v,u] = sum_c Mx[c,v] x[c,u] + row64_u   (K=65)
                    z_sb = zpool.tile([128, V], dt.float32, tag="z")
                    for h in range(2):
                        hs = slice(h * 512, (h + 1) * 512)
                        zp = pszh.tile([128, 512], dt.float32, tag="zph")
                        nc.tensor.matmul(zp[:],
                                         mxl[:, c * 128:(c + 1) * 128],
                                         xr[:, hs],
                                         start=True, stop=True)
                        nc.scalar.copy(z_sb[:, hs], zp[:])

                    # exact top-32 (values discarded, indices kept).
                    # The replace step runs off-DVE: ACT computes
                    # m=Sign(t-z) then Prelu(BIG*m-BIG, a=0.5) which maps
                    # {+1,0,-1} -> {0,-BIG/2,-BIG} (exactly 0 for z<t),
                    # and gpsimd adds it into z.  Survivor bits unchanged.
                    mv_sb = mvpool.tile([128, 32], dt.float32, tag="mv")
                    sg_sb = sgpool.tile([128, V], dt.float32, tag="sg")
                    for rnd in range(4):
                        rs = slice(rnd * 8, (rnd + 1) * 8)
                        o = c * 32 + rnd * 8
                        nc.vector.max(out=mv_sb[:, rs], in_=z_sb[:])
                        nc.vector.max_index(out=mi_sb[:, o:o + 8],
                                            in_max=mv_sb[:, rs], in_values=z_sb[:])
                        if rnd < 3:
                            t_ap = mv_sb[:, rnd * 8 + 7:rnd * 8 + 8]
                            # halves pipeline the Sign->Prelu->add chain
                            for hh in range(2):
                                h2 = slice(hh * 512, (hh + 1) * 512)
                                nc.scalar.activation(sg_sb[:, h2], z_sb[:, h2],
                                                     AF.Sign, bias=t_ap, scale=-1.0)
                                nc.scalar.activation(sg_sb[:, h2], sg_sb[:, h2],
                                                     AF.Prelu, bias=nbigc[:],
                                                     scale=bigc[:], alpha=halfc[:])
                                nc.gpsimd.tensor_tensor(out=z_sb[:, h2],
                                                        in0=z_sb[:, h2],
                                                        in1=sg_sb[:, h2], op=AO.add)
                nc.sync.dma_start(mi_d[b], mi_sb[:])

    nc.compile()
    return nc


def _get_nc():
    if "nc" not in _cache:
        _cache["nc"] = _build()
    return _cache["nc"]


def kernel(x, W, b_conv, a):
    from concourse import bass_utils

    x = np.ascontiguousarray(np.asarray(x, dtype=np.float32))
    W = np.asarray(W, dtype=np.float32)
    a = np.asarray(a, dtype=np.float32)

    nc = _get_nc()

    w = np.ascontiguousarray(W)                         # [128, 64]
    a12 = np.ascontiguousarray(
        np.stack([a[:C_REL, 0], a[C_REL:, 0]], axis=1)  # [128, 2]
    )
    xs = x.reshape(N_CORES, BPC, C_IN, V)

    in_maps = [{"x": np.ascontiguousarray(xs[c]), "w": w, "a12": a12}
               for c in range(N_CORES)]
    res = bass_utils.run_bass_kernel_spmd(nc, in_maps, list(range(N_CORES)))

    # host finish: gather r', add q', softmax over batch (constant offsets
    # q-q' and r-r' are batch-independent and cancel in the softmax)
    idx = np.empty((B, V, K), dtype=np.int64)
    q = np.empty((B, V), dtype=np.float32)
    r = np.empty((B, V), dtype=np.float32)
    for c in range(N_CORES):
        out = res.results[c]
        mi = out["mi"].reshape(BPC, 128, 8, K).transpose(0, 2, 1, 3).reshape(BPC, V, K)
        idx[c * BPC:(c + 1) * BPC] = mi
        q[c * BPC:(c + 1) * BPC] = out["qr"][:, 0, :]
        r[c * BPC:(c + 1) * BPC] = out["qr"][:, 1, :]

    pos = (np.arange(V)[:, None] * K + np.arange(K)[None, :]) % V    # [V, K]
    s = q[:, pos] + np.take_along_axis(r, idx.reshape(B, V * K), axis=1).reshape(B, V, K)
    s = s.astype(np.float32)
    m = s.max(axis=0, keepdims=True)
    e = np.exp(s - m, dtype=np.float32)
    H = e / e.sum(axis=0, keepdims=True)
    return H.astype(np.float32)
